# revision 39
# baseline (speedup 1.0000x reference)
"""Trainium2 Bass kernel for nn_BidirectionalTrustModel (histogram_binning).

Computes, per observation sequence n (N = 500000, T = 20, BINS = 12):
  1. capability edge c[n]: sequential fold over t of
       c = max(c, d)  if perf==[0,1]
       c = min(c, d)  if perf[...,0]==1
       c              otherwise
  2. trust[n] = sum_k t_k * m_k / sum_k m_k  over 12 bin centers s_k,
       m_k = (c <= s_k),  t_k = (1 + exp(beta*(dpred - s_k)))**(-zeta^2)

Key observation: trust depends on c ONLY through its bin index
b = #{k : s_k < c} (a monotone map), and monotone maps commute with the
min/max fold.  So the host recodes each (t, n) cell losslessly as a pair of
4-bit clamp params (lo, hi) in {0..11, 15} built from bucket(d) and the perf
flags — 2 bytes/cell instead of 6 — and the device scan reproduces b
EXACTLY.

Per-core layout (pure data parallel over 8 cores, no collectives):
  62500 seqs padded to 62720 = 128 partitions x 490.

Engine split per core:
  - DVE: one tensor_tensor_scan(max, min) over int8 (lo, hi) planes
    (intrinsically serial, ~2 cyc/elem), 11 tensor_scalar is_le mask ops at
    4x bf16, chunked in-place mask*T multiplies at 2x.
  - ACT (single exp/ln table, preloaded during the preamble): one
    U0 = exp(beta*dpred - beta*s_0), then L_k = ln(1 + r^k*U0) with the
    exact-f32 r^k = e^{-beta k/12} riding the Ln scale immediate (bins are
    geometric in k), one big exp(mq * L), the scan-state extraction, and
    the 1/m = exp(-ln(12 - b)) reciprocal chain (wait-hinted late so it
    cannot stall the big exp behind the scan chain).
  - PE (idle otherwise): the 12-bin masked-sum via accumulating identity
    matmuls into PSUM, interleaved with the DVE mask-mult chunks; bin 11
    (mask identically 1) is accumulated early, straight off the exp.
Scheduling: tile-major contiguous DMA blocks (multi-KB runs per
partition), wait-hints pin the static per-engine queue orders, DMA-ring
keepalives precede the (bf16) output transfer.
"""
import sys

if "/opt/trn_rl_repo" not in sys.path:
    sys.path.insert(0, "/opt/trn_rl_repo")

from contextlib import ExitStack

import ml_dtypes
import numpy as np

import concourse.bacc as bacc
import concourse.bass as bass
import concourse.mybir as mybir
import concourse.tile as tile
from concourse import bass_utils
from concourse.hw_specs import get_activation_tables as _orig_act_tables


def _combined_act_tables(arch):
    """Keep only natural_log_exp_and_others usable (positions preserved —
    the list index is the act_func_set_id) so Exp/Ln/Copy all resolve to ONE
    table: no ACT_TABLE_LOAD thrash between exp and ln."""
    t = _orig_act_tables(arch)
    return {k: (v if k == "natural_log_exp_and_others" else set())
            for k, v in t.items()}


bacc.get_activation_tables = _combined_act_tables

N_TOTAL = 500000
T = 20
BINS = 12
NCORES = 8
P = 128
F_CORE = 490
N_PAD = P * F_CORE  # 62720

AOT = mybir.AluOpType
ACTF = mybir.ActivationFunctionType
F32 = mybir.dt.float32
BF16 = mybir.dt.bfloat16
I8 = mybir.dt.int8

# scan tile widths (sequences per partition per tile); first smaller to
# prime the DMA->scan pipeline
FT = (36, 40, 88, 110, 110, 106)
# per-tile column offsets into the tile-major wp DRAM layout
FT_OFF = tuple(2 * T * sum(FT[:j]) for j in range(len(FT) + 1))
# mask-mult chunks (bin ranges) to interleave DVE mult / PE accumulate;
# bin 11 is handled separately (its mask is identically 1)
GT_CHUNKS = ((0, 2), (2, 4), (4, 6), (6, 8), (8, 10), (10, 11))


def _steps_np():
    # bit-exact match of jnp: (arange(BINS) + 0.5) / BINS in f32
    return (np.arange(BINS, dtype=np.float32) + np.float32(0.5)) / np.float32(BINS)


def build_nc(beta: float, mq: float, ncores: int = NCORES):
    p, f = P, F_CORE
    beta_f = float(np.float32(beta))
    # r^k = e^{-beta k/12}: exact f32 immediates for the per-bin Ln scale
    rk = [float(np.float32(np.exp(-np.float64(beta) * k / 12.0)))
          for k in range(BINS)]

    nc = bacc.Bacc("TRN2", target_bir_lowering=False, debug=False,
                   enable_asserts=False, num_devices=ncores)

    # tile-major layout: tile j occupies contiguous columns
    # [FT_OFF[j], FT_OFF[j+1]) = [lo-plane | hi-plane] of that tile, so each
    # per-tile DMA is one 2D copy with a multi-KB contiguous run per
    # partition (20B runs would cripple DMA efficiency)
    d_wp = nc.dram_tensor("wp", [p, 2 * f * T], I8, kind="ExternalInput").ap()
    d_dpred = nc.dram_tensor("dpred", [N_PAD], F32, kind="ExternalInput").ap()
    d_wm = nc.dram_tensor("wmats", [p, 128], BF16,
                          kind="ExternalInput").ap()
    d_cb = nc.dram_tensor("cbias", [p, 2], F32, kind="ExternalInput").ap()
    d_out = nc.dram_tensor("out", [p, f], BF16, kind="ExternalOutput").ap()
    d_scr = nc.dram_tensor("dscr", [p, f], BF16, kind="Internal").ap()

    with tile.TileContext(nc) as tc:
        with ExitStack() as ctx:
            inpool = ctx.enter_context(tc.tile_pool(name="in",
                                                    bufs=min(4, len(FT))))
            keep = ctx.enter_context(tc.tile_pool(name="keep", bufs=1))
            psum_s = ctx.enter_context(tc.tile_pool(name="psS", bufs=1,
                                                    space="PSUM"))

            DP = keep.tile([p, f], F32, tag="DP")
            WT = keep.tile([p, 128], BF16, tag="WT")
            CS = keep.tile([p, f * T], BF16, tag="CS")
            U0 = keep.tile([p, f], F32, tag="U0")
            L = keep.tile([p, BINS * f], F32, tag="L")
            Tt = keep.tile([p, BINS * f], BF16, tag="Tt")
            G = keep.tile([p, BINS * f], BF16, tag="G")
            C = keep.tile([p, f], BF16, tag="C")
            LM = keep.tile([p, f], F32, tag="LM")
            REC = keep.tile([p, f], F32, tag="REC")
            OUT = keep.tile([p, f], BF16, tag="OUT")
            CBt = keep.tile([p, 2], F32, tag="CBt")
            DUM = keep.tile([p, 1], F32, tag="DUM")

            # warm the ACT table during the preamble window: memset + tiny
            # exp forces the one ACT_TABLE_LOAD before any real dependency
            nc.gpsimd.memset(DUM[:], 0.0)
            nc.scalar.activation(DUM[:], DUM[:], ACTF.Exp)

            # aux DMAs: small early ones on the gpsimd queue, the 32KB
            # identity on the scalar queue (wp tiles own the sync queue)
            nc.gpsimd.dma_start(CBt[:], d_cb)
            nc.gpsimd.dma_start(DP[:], d_dpred.rearrange("(p n) -> p n", p=p))
            nc.scalar.dma_start(WT[:], d_wm)

            # U0 = exp(beta*dpred - beta*s_0)  [p, f] f32
            nc.scalar.activation(U0[:], DP[:], ACTF.Exp, bias=CBt[:, 0:1],
                                 scale=beta_f)

            # L_k = ln(1 + r^k * U0): bins are geometric in k, so one exp
            # feeds all 12 bins; r^k rides the Ln's (exact f32) scale imm
            for k in range(BINS):
                nc.scalar.activation(L[:, f * k: f * (k + 1)], U0[:],
                                     ACTF.Ln, bias=1.0, scale=rk[k])

            # T = exp(mq * L)  [p, 12f] bf16
            nc.scalar.activation(Tt[:], L[:], ACTF.Exp,
                                 scale=float(np.float32(mq)))

            # bin 11 survives every mask (b <= 11 always): accumulate its
            # unmasked T slab into PSUM right away, long before the scan ends
            TS = psum_s.tile([p, 512], F32, tag="TS")
            nc.tensor.matmul(TS[:, :f], WT[:, 0:128],
                             Tt[:, f * 11: f * 12], start=True, stop=False)

            # phase A: pipelined DMA + scan over (lo, hi) int8 planes.
            # slot-0 carries lo=hi=v0 so state = v0 exactly regardless of
            # the carry-in: sequences pack back-to-back in ONE flat stream.
            base = 0
            for j, ftj in enumerate(FT):
                FT20 = ftj * T
                WPt = inpool.tile([p, 2 * FT20], I8, tag="WPt")
                nc.sync.dma_start(WPt[:], d_wp[:, FT_OFF[j]:FT_OFF[j + 1]])
                nc.vector.tensor_tensor_scan(CS[:, T * base: T * (base + ftj)],
                                             WPt[:, 0:FT20], WPt[:, FT20:],
                                             0.0, AOT.max, AOT.min)
                base += ftj

            # b per sequence = scan state at t = T-1; extracted on ACT
            # (idle by then) in two pieces so the DVE jumps straight from
            # the last scan into the mask chain
            f_a = sum(FT[:-1])
            cs_v = CS[:].rearrange("p (n t) -> p n t", t=T)[:, :, T - 1]
            with tc.tile_wait_until(0.018):
                nc.scalar.activation(C[:, 0:f_a], cs_v[:, 0:f_a], ACTF.Copy)
            nc.vector.tensor_copy(C[:, f_a:f], cs_v[:, f_a:f])

            # 1/m = exp(-ln(12 - b)) on ACT — wait-hinted late so the
            # static ACT queue order keeps it AFTER the big exp above
            with tc.tile_wait_until(0.020):
                nc.scalar.activation(LM[:], C[:], ACTF.Ln, bias=CBt[:, 1:2],
                                     scale=-1.0)
                nc.scalar.activation(REC[:], LM[:], ACTF.Exp, scale=-1.0)

            # tail: per chunk, G_k = (b <= k) masks (4x tensor_scalar),
            # masked mult (2x), then PE PSUM accumulation — interleaved so
            # the PE starts as early as possible (bin 11 needs no mask)
            for c, (ka, kb) in enumerate(GT_CHUNKS):
                with tc.tile_wait_until(0.020 + 0.0004 * c):
                    for k in range(ka, kb):
                        nc.vector.tensor_scalar(G[:, f * k: f * (k + 1)],
                                                C[:], float(k), None,
                                                AOT.is_le)
                    sl = slice(f * ka, f * kb)
                    nc.vector.tensor_tensor(G[:, sl], G[:, sl], Tt[:, sl],
                                            AOT.mult)
                with tc.tile_wait_until(0.0202 + 0.0004 * c):
                    for k in range(ka, kb):
                        nc.tensor.matmul(TS[:, :f], WT[:, 0:128],
                                         G[:, f * k: f * (k + 1)],
                                         start=False,
                                         stop=(k == BINS - 2))

            # trust = tsum * (1/m)
            nc.vector.tensor_tensor(OUT[:], TS[:, :f], REC[:], AOT.mult)
            nc.sync.dma_start(d_out, OUT[:])

    nc.compile()
    return nc


_CACHE: dict = {}


def _get_nc(beta: float, mq: float):
    key = (beta, mq)
    if key not in _CACHE:
        _CACHE[key] = build_nc(beta, mq)
    return _CACHE[key]


def make_in_maps(inptasksperf, difficulties_obs, difficulties_pred,
                 n_total=N_TOTAL, ncores=NCORES, n_pad=N_PAD, p=P):
    """Host-side recode: bucket(d_obs) + perf flags -> (lo, hi) int8 clamp
    params, shard + pad + t-inner relayout."""
    perf = np.asarray(inptasksperf)
    dobs = np.asarray(difficulties_obs, dtype=np.float32)[..., 0]    # [T, N]
    dpred = np.asarray(difficulties_pred, dtype=np.float32)[..., 0]  # [N]
    f = n_pad // p
    nc_n = n_total // ncores
    steps = _steps_np()

    # b = #{k : s_k < d} in 0..11 (exact f32 comparisons, matches the
    # reference's mask since bucketing commutes with the min/max fold)
    b = np.searchsorted(steps, dobs.ravel(), side="left").astype(
        np.int8).reshape(dobs.shape)
    p0 = perf[..., 0] != 0
    p1 = perf[..., 1] != 0
    suc = p1 & ~p0
    lo = np.where(suc, b, 0).astype(np.int8)
    hi = np.where(p0, b, np.int8(15)).astype(np.int8)
    # slot-0 self-reset: state after step 0 is exactly v0
    v0 = np.where(suc[0], b[0], 0).astype(np.int8)
    lo[0] = v0
    hi[0] = v0

    in_maps = []
    for c in range(ncores):
        sl = slice(c * nc_n, (c + 1) * nc_n)
        lop = np.zeros((T, n_pad), np.int8)
        lop[:, :nc_n] = lo[:, sl]
        hip = np.zeros((T, n_pad), np.int8)
        hip[:, :nc_n] = hi[:, sl]
        loc = lop.reshape(T, p, f).transpose(1, 2, 0)   # [p, f, T]
        hic = hip.reshape(T, p, f).transpose(1, 2, 0)
        # tile-major pack: [lo-tile | hi-tile] contiguous per scan tile
        blocks = []
        fa = 0
        for ftj in FT:
            blocks.append(loc[:, fa:fa + ftj, :].reshape(p, ftj * T))
            blocks.append(hic[:, fa:fa + ftj, :].reshape(p, ftj * T))
            fa += ftj
        wp = np.ascontiguousarray(np.concatenate(blocks, axis=1))  # [p,2fT]

        dpc = np.zeros((n_pad,), np.float32)
        dpc[:nc_n] = dpred[sl]
        in_maps.append({"wp": wp, "dpred": dpc})
    return in_maps


def make_consts(beta, p=P):
    """Identity (bf16) for the PE PSUM-accumulate, plus activation bias
    consts."""
    wm = np.eye(p, 128, dtype=np.float32)
    steps = _steps_np()
    cb0 = np.float32(-(np.float64(beta) * np.float64(steps[0])))
    cb = np.broadcast_to(np.array([cb0, np.float32(BINS)], np.float32),
                         (p, 2))
    return {"wmats": wm.astype(ml_dtypes.bfloat16),
            "cbias": np.ascontiguousarray(cb)}


def kernel(inptasksobs=None, inptasksperf=None, inptaskspred=None,
           num_obs_tasks=None, tasksobsids=None, taskspredids=None,
           difficulties_obs=None, difficulties_pred=None,
           betas=None, zetas=None, **_):
    beta = float(np.float32(np.asarray(betas).reshape(-1)[0]))
    zeta = np.float32(np.asarray(zetas).reshape(-1)[0])
    mq = float(np.float32(-(zeta * zeta)))

    nc = _get_nc(beta, mq)
    in_maps = make_in_maps(inptasksperf, difficulties_obs, difficulties_pred)
    consts = make_consts(beta)
    for m in in_maps:
        m.update(consts)
    res = bass_utils.run_bass_kernel_spmd(nc, in_maps,
                                          core_ids=list(range(NCORES)))
    nc_n = N_TOTAL // NCORES
    parts = [np.asarray(r["out"]).reshape(-1)[:nc_n] for r in res.results]
    return np.concatenate(parts).reshape(N_TOTAL, 1).astype(np.float32)


if __name__ == "__main__":
    rng = np.random.default_rng(0)
    ins = {
        "inptasksperf": rng.integers(0, 2, (T, N_TOTAL, 2)).astype(np.int32),
        "difficulties_obs": (0.9 * rng.random((T, N_TOTAL, 1))).astype(np.float32),
        "difficulties_pred": (0.9 * rng.random((N_TOTAL, 1))).astype(np.float32),
        "betas": np.array([7.0], np.float32),
        "zetas": np.array([0.5], np.float32),
    }
    out = kernel(**ins)
    print(out.shape, out.dtype, out[:5, 0])


# revision 40
# speedup vs baseline: 1.0141x; 1.0141x over previous
"""Trainium2 Bass kernel for nn_BidirectionalTrustModel (histogram_binning).

Computes, per observation sequence n (N = 500000, T = 20, BINS = 12):
  1. capability edge c[n]: sequential fold over t of
       c = max(c, d)  if perf==[0,1]
       c = min(c, d)  if perf[...,0]==1
       c              otherwise
  2. trust[n] = sum_k t_k * m_k / sum_k m_k  over 12 bin centers s_k,
       m_k = (c <= s_k),  t_k = (1 + exp(beta*(dpred - s_k)))**(-zeta^2)

Key observation: trust depends on c ONLY through its bin index
b = #{k : s_k < c} (a monotone map), and monotone maps commute with the
min/max fold.  So the host recodes each (t, n) cell losslessly as a pair of
4-bit clamp params (lo, hi) in {0..11, 15} built from bucket(d) and the perf
flags — 2 bytes/cell instead of 6 — and the device scan reproduces b
EXACTLY.

Per-core layout (pure data parallel over 8 cores, no collectives):
  62500 seqs padded to 62720 = 128 partitions x 490.

Engine split per core:
  - DVE: one tensor_tensor_scan(max, min) over int8 (lo, hi) planes
    (intrinsically serial, ~2 cyc/elem), 11 tensor_scalar is_le mask ops at
    4x bf16, chunked in-place mask*T multiplies at 2x.
  - ACT (single exp/ln table, preloaded during the preamble): one
    U0 = exp(beta*dpred - beta*s_0), then L_k = ln(1 + r^k*U0) with the
    exact-f32 r^k = e^{-beta k/12} riding the Ln scale immediate (bins are
    geometric in k), one big exp(mq * L), the scan-state extraction, and
    the 1/m = exp(-ln(12 - b)) reciprocal chain (wait-hinted late so it
    cannot stall the big exp behind the scan chain).
  - PE (idle otherwise): the 12-bin masked-sum via accumulating identity
    matmuls into PSUM, interleaved with the DVE mask-mult chunks; bin 11
    (mask identically 1) is accumulated early, straight off the exp.
Scheduling: tile-major contiguous DMA blocks (multi-KB runs per
partition), wait-hints pin the static per-engine queue orders, DMA-ring
keepalives precede the (bf16) output transfer.
"""
import sys

if "/opt/trn_rl_repo" not in sys.path:
    sys.path.insert(0, "/opt/trn_rl_repo")

from contextlib import ExitStack

import ml_dtypes
import numpy as np

import concourse.bacc as bacc
import concourse.bass as bass
import concourse.mybir as mybir
import concourse.tile as tile
from concourse import bass_utils
from concourse.hw_specs import get_activation_tables as _orig_act_tables


def _combined_act_tables(arch):
    """Keep only natural_log_exp_and_others usable (positions preserved —
    the list index is the act_func_set_id) so Exp/Ln/Copy all resolve to ONE
    table: no ACT_TABLE_LOAD thrash between exp and ln."""
    t = _orig_act_tables(arch)
    return {k: (v if k == "natural_log_exp_and_others" else set())
            for k, v in t.items()}


bacc.get_activation_tables = _combined_act_tables

N_TOTAL = 500000
T = 20
BINS = 12
NCORES = 8
P = 128
F_CORE = 490
N_PAD = P * F_CORE  # 62720

AOT = mybir.AluOpType
ACTF = mybir.ActivationFunctionType
F32 = mybir.dt.float32
BF16 = mybir.dt.bfloat16
I8 = mybir.dt.int8

# scan tile widths (sequences per partition per tile); first smaller to
# prime the DMA->scan pipeline
FT = (30, 36, 94, 110, 110, 110)
# per-tile column offsets into the tile-major wp DRAM layout
FT_OFF = tuple(2 * T * sum(FT[:j]) for j in range(len(FT) + 1))
# mask-mult chunks (bin ranges) to interleave DVE mult / PE accumulate;
# bin 11 is handled separately (its mask is identically 1)
GT_CHUNKS = ((0, 2), (2, 4), (4, 6), (6, 8), (8, 10), (10, 11))


def _steps_np():
    # bit-exact match of jnp: (arange(BINS) + 0.5) / BINS in f32
    return (np.arange(BINS, dtype=np.float32) + np.float32(0.5)) / np.float32(BINS)


def build_nc(beta: float, mq: float, ncores: int = NCORES):
    p, f = P, F_CORE
    beta_f = float(np.float32(beta))
    # r^k = e^{-beta k/12}: exact f32 immediates for the per-bin Ln scale
    rk = [float(np.float32(np.exp(-np.float64(beta) * k / 12.0)))
          for k in range(BINS)]

    nc = bacc.Bacc("TRN2", target_bir_lowering=False, debug=False,
                   enable_asserts=False, num_devices=ncores)

    # tile-major layout: tile j occupies contiguous columns
    # [FT_OFF[j], FT_OFF[j+1]) = [lo-plane | hi-plane] of that tile, so each
    # per-tile DMA is one 2D copy with a multi-KB contiguous run per
    # partition (20B runs would cripple DMA efficiency)
    d_wp = nc.dram_tensor("wp", [p, 2 * f * T], I8, kind="ExternalInput").ap()
    d_dpred = nc.dram_tensor("dpred", [N_PAD], F32, kind="ExternalInput").ap()
    d_wm = nc.dram_tensor("wmats", [p, 128], BF16,
                          kind="ExternalInput").ap()
    d_cb = nc.dram_tensor("cbias", [p, 2], F32, kind="ExternalInput").ap()
    d_out = nc.dram_tensor("out", [p, f], BF16, kind="ExternalOutput").ap()
    d_scr = nc.dram_tensor("dscr", [p, f], BF16, kind="Internal").ap()

    with tile.TileContext(nc) as tc:
        with ExitStack() as ctx:
            inpool = ctx.enter_context(tc.tile_pool(name="in",
                                                    bufs=min(4, len(FT))))
            keep = ctx.enter_context(tc.tile_pool(name="keep", bufs=1))
            psum_s = ctx.enter_context(tc.tile_pool(name="psS", bufs=1,
                                                    space="PSUM"))

            DP = keep.tile([p, f], F32, tag="DP")
            WT = keep.tile([p, 128], BF16, tag="WT")
            CS = keep.tile([p, f * T], BF16, tag="CS")
            U0 = keep.tile([p, f], F32, tag="U0")
            L = keep.tile([p, BINS * f], F32, tag="L")
            Tt = keep.tile([p, BINS * f], BF16, tag="Tt")
            G = keep.tile([p, BINS * f], BF16, tag="G")
            C = keep.tile([p, f], BF16, tag="C")
            LM = keep.tile([p, f], F32, tag="LM")
            REC = keep.tile([p, f], F32, tag="REC")
            OUT = keep.tile([p, f], BF16, tag="OUT")
            CBt = keep.tile([p, 2], F32, tag="CBt")
            DUM = keep.tile([p, 1], F32, tag="DUM")

            # warm the ACT table during the preamble window: memset + tiny
            # exp forces the one ACT_TABLE_LOAD before any real dependency
            nc.gpsimd.memset(DUM[:], 0.0)
            nc.scalar.activation(DUM[:], DUM[:], ACTF.Exp)

            # aux DMAs: small early ones on the gpsimd queue, the 32KB
            # identity on the scalar queue (wp tiles own the sync queue)
            nc.gpsimd.dma_start(CBt[:], d_cb)
            nc.gpsimd.dma_start(DP[:], d_dpred.rearrange("(p n) -> p n", p=p))
            nc.scalar.dma_start(WT[:], d_wm)

            # U0 = exp(beta*dpred - beta*s_0)  [p, f] f32
            nc.scalar.activation(U0[:], DP[:], ACTF.Exp, bias=CBt[:, 0:1],
                                 scale=beta_f)

            # L_k = ln(1 + r^k * U0): bins are geometric in k, so one exp
            # feeds all 12 bins; r^k rides the Ln's (exact f32) scale imm
            for k in range(BINS):
                nc.scalar.activation(L[:, f * k: f * (k + 1)], U0[:],
                                     ACTF.Ln, bias=1.0, scale=rk[k])

            # T = exp(mq * L)  [p, 12f] bf16
            nc.scalar.activation(Tt[:], L[:], ACTF.Exp,
                                 scale=float(np.float32(mq)))

            # bin 11 survives every mask (b <= 11 always): accumulate its
            # unmasked T slab into PSUM right away, long before the scan ends
            TS = psum_s.tile([p, 512], F32, tag="TS")
            nc.tensor.matmul(TS[:, :f], WT[:, 0:128],
                             Tt[:, f * 11: f * 12], start=True, stop=False)

            # phase A: pipelined DMA + scan over (lo, hi) int8 planes.
            # slot-0 carries lo=hi=v0 so state = v0 exactly regardless of
            # the carry-in: sequences pack back-to-back in ONE flat stream.
            base = 0
            for j, ftj in enumerate(FT):
                FT20 = ftj * T
                WPt = inpool.tile([p, 2 * FT20], I8, tag="WPt")
                nc.sync.dma_start(WPt[:], d_wp[:, FT_OFF[j]:FT_OFF[j + 1]])
                nc.vector.tensor_tensor_scan(CS[:, T * base: T * (base + ftj)],
                                             WPt[:, 0:FT20], WPt[:, FT20:],
                                             0.0, AOT.max, AOT.min)
                base += ftj

            # b per sequence = scan state at t = T-1; extracted on ACT
            # (idle by then) in two pieces so the DVE jumps straight from
            # the last scan into the mask chain
            f_a = sum(FT[:-1])
            cs_v = CS[:].rearrange("p (n t) -> p n t", t=T)[:, :, T - 1]
            with tc.tile_wait_until(0.018):
                nc.scalar.activation(C[:, 0:f_a], cs_v[:, 0:f_a], ACTF.Copy)
            nc.vector.tensor_copy(C[:, f_a:f], cs_v[:, f_a:f])

            # 1/m = exp(-ln(12 - b)) on ACT — wait-hinted late so the
            # static ACT queue order keeps it AFTER the big exp above
            with tc.tile_wait_until(0.020):
                nc.scalar.activation(LM[:], C[:], ACTF.Ln, bias=CBt[:, 1:2],
                                     scale=-1.0)
                nc.scalar.activation(REC[:], LM[:], ACTF.Exp, scale=-1.0)

            # tail: per chunk, G_k = (b <= k) masks (4x tensor_scalar),
            # masked mult (2x), then PE PSUM accumulation — interleaved so
            # the PE starts as early as possible (bin 11 needs no mask)
            for c, (ka, kb) in enumerate(GT_CHUNKS):
                with tc.tile_wait_until(0.020 + 0.0004 * c):
                    for k in range(ka, kb):
                        nc.vector.tensor_scalar(G[:, f * k: f * (k + 1)],
                                                C[:], float(k), None,
                                                AOT.is_le)
                    sl = slice(f * ka, f * kb)
                    nc.vector.tensor_tensor(G[:, sl], G[:, sl], Tt[:, sl],
                                            AOT.mult)
                with tc.tile_wait_until(0.0202 + 0.0004 * c):
                    for k in range(ka, kb):
                        nc.tensor.matmul(TS[:, :f], WT[:, 0:128],
                                         G[:, f * k: f * (k + 1)],
                                         start=False,
                                         stop=(k == BINS - 2))

            # trust = tsum * (1/m)
            nc.vector.tensor_tensor(OUT[:], TS[:, :f], REC[:], AOT.mult)
            nc.sync.dma_start(d_out, OUT[:])

    nc.compile()
    return nc


_CACHE: dict = {}


def _get_nc(beta: float, mq: float):
    key = (beta, mq)
    if key not in _CACHE:
        _CACHE[key] = build_nc(beta, mq)
    return _CACHE[key]


def make_in_maps(inptasksperf, difficulties_obs, difficulties_pred,
                 n_total=N_TOTAL, ncores=NCORES, n_pad=N_PAD, p=P):
    """Host-side recode: bucket(d_obs) + perf flags -> (lo, hi) int8 clamp
    params, shard + pad + t-inner relayout."""
    perf = np.asarray(inptasksperf)
    dobs = np.asarray(difficulties_obs, dtype=np.float32)[..., 0]    # [T, N]
    dpred = np.asarray(difficulties_pred, dtype=np.float32)[..., 0]  # [N]
    f = n_pad // p
    nc_n = n_total // ncores
    steps = _steps_np()

    # b = #{k : s_k < d} in 0..11 (exact f32 comparisons, matches the
    # reference's mask since bucketing commutes with the min/max fold)
    b = np.searchsorted(steps, dobs.ravel(), side="left").astype(
        np.int8).reshape(dobs.shape)
    p0 = perf[..., 0] != 0
    p1 = perf[..., 1] != 0
    suc = p1 & ~p0
    lo = np.where(suc, b, 0).astype(np.int8)
    hi = np.where(p0, b, np.int8(15)).astype(np.int8)
    # slot-0 self-reset: state after step 0 is exactly v0
    v0 = np.where(suc[0], b[0], 0).astype(np.int8)
    lo[0] = v0
    hi[0] = v0

    in_maps = []
    for c in range(ncores):
        sl = slice(c * nc_n, (c + 1) * nc_n)
        lop = np.zeros((T, n_pad), np.int8)
        lop[:, :nc_n] = lo[:, sl]
        hip = np.zeros((T, n_pad), np.int8)
        hip[:, :nc_n] = hi[:, sl]
        loc = lop.reshape(T, p, f).transpose(1, 2, 0)   # [p, f, T]
        hic = hip.reshape(T, p, f).transpose(1, 2, 0)
        # tile-major pack: [lo-tile | hi-tile] contiguous per scan tile
        blocks = []
        fa = 0
        for ftj in FT:
            blocks.append(loc[:, fa:fa + ftj, :].reshape(p, ftj * T))
            blocks.append(hic[:, fa:fa + ftj, :].reshape(p, ftj * T))
            fa += ftj
        wp = np.ascontiguousarray(np.concatenate(blocks, axis=1))  # [p,2fT]

        dpc = np.zeros((n_pad,), np.float32)
        dpc[:nc_n] = dpred[sl]
        in_maps.append({"wp": wp, "dpred": dpc})
    return in_maps


def make_consts(beta, p=P):
    """Identity (bf16) for the PE PSUM-accumulate, plus activation bias
    consts."""
    wm = np.eye(p, 128, dtype=np.float32)
    steps = _steps_np()
    cb0 = np.float32(-(np.float64(beta) * np.float64(steps[0])))
    cb = np.broadcast_to(np.array([cb0, np.float32(BINS)], np.float32),
                         (p, 2))
    return {"wmats": wm.astype(ml_dtypes.bfloat16),
            "cbias": np.ascontiguousarray(cb)}


def kernel(inptasksobs=None, inptasksperf=None, inptaskspred=None,
           num_obs_tasks=None, tasksobsids=None, taskspredids=None,
           difficulties_obs=None, difficulties_pred=None,
           betas=None, zetas=None, **_):
    beta = float(np.float32(np.asarray(betas).reshape(-1)[0]))
    zeta = np.float32(np.asarray(zetas).reshape(-1)[0])
    mq = float(np.float32(-(zeta * zeta)))

    nc = _get_nc(beta, mq)
    in_maps = make_in_maps(inptasksperf, difficulties_obs, difficulties_pred)
    consts = make_consts(beta)
    for m in in_maps:
        m.update(consts)
    res = bass_utils.run_bass_kernel_spmd(nc, in_maps,
                                          core_ids=list(range(NCORES)))
    nc_n = N_TOTAL // NCORES
    parts = [np.asarray(r["out"]).reshape(-1)[:nc_n] for r in res.results]
    return np.concatenate(parts).reshape(N_TOTAL, 1).astype(np.float32)


if __name__ == "__main__":
    rng = np.random.default_rng(0)
    ins = {
        "inptasksperf": rng.integers(0, 2, (T, N_TOTAL, 2)).astype(np.int32),
        "difficulties_obs": (0.9 * rng.random((T, N_TOTAL, 1))).astype(np.float32),
        "difficulties_pred": (0.9 * rng.random((N_TOTAL, 1))).astype(np.float32),
        "betas": np.array([7.0], np.float32),
        "zetas": np.array([0.5], np.float32),
    }
    out = kernel(**ins)
    print(out.shape, out.dtype, out[:5, 0])


# revision 41
# speedup vs baseline: 1.0191x; 1.0049x over previous
"""Trainium2 Bass kernel for nn_BidirectionalTrustModel (histogram_binning).

Computes, per observation sequence n (N = 500000, T = 20, BINS = 12):
  1. capability edge c[n]: sequential fold over t of
       c = max(c, d)  if perf==[0,1]
       c = min(c, d)  if perf[...,0]==1
       c              otherwise
  2. trust[n] = sum_k t_k * m_k / sum_k m_k  over 12 bin centers s_k,
       m_k = (c <= s_k),  t_k = (1 + exp(beta*(dpred - s_k)))**(-zeta^2)

Key observation: trust depends on c ONLY through its bin index
b = #{k : s_k < c} (a monotone map), and monotone maps commute with the
min/max fold.  So the host recodes each (t, n) cell losslessly as a pair of
4-bit clamp params (lo, hi) in {0..11, 15} built from bucket(d) and the perf
flags — 2 bytes/cell instead of 6 — and the device scan reproduces b
EXACTLY.

Per-core layout (pure data parallel over 8 cores, no collectives):
  62500 seqs padded to 62720 = 128 partitions x 490.

Engine split per core:
  - DVE: one tensor_tensor_scan(max, min) over int8 (lo, hi) planes
    (intrinsically serial, ~2 cyc/elem), 11 tensor_scalar is_le mask ops at
    4x bf16, chunked in-place mask*T multiplies at 2x.
  - ACT (single exp/ln table, preloaded during the preamble): one
    U0 = exp(beta*dpred - beta*s_0), then L_k = ln(1 + r^k*U0) with the
    exact-f32 r^k = e^{-beta k/12} riding the Ln scale immediate (bins are
    geometric in k), one big exp(mq * L), the scan-state extraction, and
    the 1/m = exp(-ln(12 - b)) reciprocal chain (wait-hinted late so it
    cannot stall the big exp behind the scan chain).
  - PE (idle otherwise): the 12-bin masked-sum via accumulating identity
    matmuls into PSUM, interleaved with the DVE mask-mult chunks; bin 11
    (mask identically 1) is accumulated early, straight off the exp.
Scheduling: tile-major contiguous DMA blocks (multi-KB runs per
partition); wait-hints pin the static per-engine queue orders; bf16
output transfer.
"""
import sys

if "/opt/trn_rl_repo" not in sys.path:
    sys.path.insert(0, "/opt/trn_rl_repo")

from contextlib import ExitStack

import ml_dtypes
import numpy as np

import concourse.bacc as bacc
import concourse.bass as bass
import concourse.mybir as mybir
import concourse.tile as tile
from concourse import bass_utils
from concourse.hw_specs import get_activation_tables as _orig_act_tables


def _combined_act_tables(arch):
    """Keep only natural_log_exp_and_others usable (positions preserved —
    the list index is the act_func_set_id) so Exp/Ln/Copy all resolve to ONE
    table: no ACT_TABLE_LOAD thrash between exp and ln."""
    t = _orig_act_tables(arch)
    return {k: (v if k == "natural_log_exp_and_others" else set())
            for k, v in t.items()}


bacc.get_activation_tables = _combined_act_tables

N_TOTAL = 500000
T = 20
BINS = 12
NCORES = 8
P = 128
F_CORE = 490
N_PAD = P * F_CORE  # 62720

AOT = mybir.AluOpType
ACTF = mybir.ActivationFunctionType
F32 = mybir.dt.float32
BF16 = mybir.dt.bfloat16
I8 = mybir.dt.int8

# scan tile widths (sequences per partition per tile); first smaller to
# prime the DMA->scan pipeline
FT = (30, 36, 94, 110, 110, 110)
# per-tile column offsets into the tile-major wp DRAM layout
FT_OFF = tuple(2 * T * sum(FT[:j]) for j in range(len(FT) + 1))
# mask-mult chunks (bin ranges) to interleave DVE mult / PE accumulate;
# bin 11 is handled separately (its mask is identically 1)
GT_CHUNKS = ((0, 2), (2, 4), (4, 6), (6, 8), (8, 10), (10, 11))


def _steps_np():
    # bit-exact match of jnp: (arange(BINS) + 0.5) / BINS in f32
    return (np.arange(BINS, dtype=np.float32) + np.float32(0.5)) / np.float32(BINS)


def build_nc(beta: float, mq: float, ncores: int = NCORES):
    p, f = P, F_CORE
    beta_f = float(np.float32(beta))
    # r^k = e^{-beta k/12}: exact f32 immediates for the per-bin Ln scale
    rk = [float(np.float32(np.exp(-np.float64(beta) * k / 12.0)))
          for k in range(BINS)]

    nc = bacc.Bacc("TRN2", target_bir_lowering=False, debug=False,
                   enable_asserts=False, num_devices=ncores)

    # tile-major layout: tile j occupies contiguous columns
    # [FT_OFF[j], FT_OFF[j+1]) = [lo-plane | hi-plane] of that tile, so each
    # per-tile DMA is one 2D copy with a multi-KB contiguous run per
    # partition (20B runs would cripple DMA efficiency)
    d_wp = nc.dram_tensor("wp", [p, 2 * f * T], I8, kind="ExternalInput").ap()
    d_dpred = nc.dram_tensor("dpred", [N_PAD], F32, kind="ExternalInput").ap()
    d_wm = nc.dram_tensor("wmats", [p, 128], BF16,
                          kind="ExternalInput").ap()
    d_cb = nc.dram_tensor("cbias", [p, 2], F32, kind="ExternalInput").ap()
    d_out = nc.dram_tensor("out", [p, f], BF16, kind="ExternalOutput").ap()

    with tile.TileContext(nc) as tc:
        with ExitStack() as ctx:
            inpool = ctx.enter_context(tc.tile_pool(name="in",
                                                    bufs=min(4, len(FT))))
            keep = ctx.enter_context(tc.tile_pool(name="keep", bufs=1))
            psum_s = ctx.enter_context(tc.tile_pool(name="psS", bufs=1,
                                                    space="PSUM"))

            DP = keep.tile([p, f], F32, tag="DP")
            WT = keep.tile([p, 128], BF16, tag="WT")
            CS = keep.tile([p, f * T], BF16, tag="CS")
            U0 = keep.tile([p, f], F32, tag="U0")
            L = keep.tile([p, BINS * f], F32, tag="L")
            Tt = keep.tile([p, BINS * f], BF16, tag="Tt")
            G = keep.tile([p, BINS * f], BF16, tag="G")
            C = keep.tile([p, f], BF16, tag="C")
            LM = keep.tile([p, f], F32, tag="LM")
            REC = keep.tile([p, f], F32, tag="REC")
            OUT = keep.tile([p, f], BF16, tag="OUT")
            CBt = keep.tile([p, 2], F32, tag="CBt")
            DUM = keep.tile([p, 1], F32, tag="DUM")

            # warm the ACT table during the preamble window: memset + tiny
            # exp forces the one ACT_TABLE_LOAD before any real dependency
            nc.gpsimd.memset(DUM[:], 0.0)
            nc.scalar.activation(DUM[:], DUM[:], ACTF.Exp)

            # aux DMAs: small early ones on the gpsimd queue, the 32KB
            # identity on the scalar queue (wp tiles own the sync queue)
            nc.gpsimd.dma_start(CBt[:], d_cb)
            nc.gpsimd.dma_start(DP[:], d_dpred.rearrange("(p n) -> p n", p=p))
            nc.scalar.dma_start(WT[:], d_wm)

            # U0 = exp(beta*dpred - beta*s_0)  [p, f] f32
            nc.scalar.activation(U0[:], DP[:], ACTF.Exp, bias=CBt[:, 0:1],
                                 scale=beta_f)

            # L_k = ln(1 + r^k * U0): bins are geometric in k, so one exp
            # feeds all 12 bins; r^k rides the Ln's (exact f32) scale imm
            for k in range(BINS):
                nc.scalar.activation(L[:, f * k: f * (k + 1)], U0[:],
                                     ACTF.Ln, bias=1.0, scale=rk[k])

            # T = exp(mq * L)  [p, 12f] bf16
            nc.scalar.activation(Tt[:], L[:], ACTF.Exp,
                                 scale=float(np.float32(mq)))

            # bin 11 survives every mask (b <= 11 always): accumulate its
            # unmasked T slab into PSUM right away, long before the scan ends
            TS = psum_s.tile([p, 512], F32, tag="TS")
            nc.tensor.matmul(TS[:, :f], WT[:, 0:128],
                             Tt[:, f * 11: f * 12], start=True, stop=False)

            # phase A: pipelined DMA + scan over (lo, hi) int8 planes.
            # slot-0 carries lo=hi=v0 so state = v0 exactly regardless of
            # the carry-in: sequences pack back-to-back in ONE flat stream.
            base = 0
            for j, ftj in enumerate(FT):
                FT20 = ftj * T
                WPt = inpool.tile([p, 2 * FT20], I8, tag="WPt")
                nc.sync.dma_start(WPt[:], d_wp[:, FT_OFF[j]:FT_OFF[j + 1]])
                nc.vector.tensor_tensor_scan(CS[:, T * base: T * (base + ftj)],
                                             WPt[:, 0:FT20], WPt[:, FT20:],
                                             0.0, AOT.max, AOT.min)
                base += ftj

            # b per sequence = scan state at t = T-1; extracted on ACT
            # (idle by then) in two pieces so the DVE jumps straight from
            # the last scan into the mask chain
            f_a = sum(FT[:-1])
            cs_v = CS[:].rearrange("p (n t) -> p n t", t=T)[:, :, T - 1]
            with tc.tile_wait_until(0.018):
                nc.scalar.activation(C[:, 0:f_a], cs_v[:, 0:f_a], ACTF.Copy)
            nc.vector.tensor_copy(C[:, f_a:f], cs_v[:, f_a:f])

            # 1/m = exp(-ln(12 - b)) on ACT — wait-hinted late so the
            # static ACT queue order keeps it AFTER the big exp above
            with tc.tile_wait_until(0.020):
                nc.scalar.activation(LM[:], C[:], ACTF.Ln, bias=CBt[:, 1:2],
                                     scale=-1.0)
                nc.scalar.activation(REC[:], LM[:], ACTF.Exp, scale=-1.0)

            # tail: per chunk, G_k = (b <= k) masks (4x tensor_scalar),
            # masked mult (2x), then PE PSUM accumulation — interleaved so
            # the PE starts as early as possible (bin 11 needs no mask)
            for c, (ka, kb) in enumerate(GT_CHUNKS):
                with tc.tile_wait_until(0.020 + 0.0004 * c):
                    for k in range(ka, kb):
                        nc.vector.tensor_scalar(G[:, f * k: f * (k + 1)],
                                                C[:], float(k), None,
                                                AOT.is_le)
                    sl = slice(f * ka, f * kb)
                    nc.vector.tensor_tensor(G[:, sl], G[:, sl], Tt[:, sl],
                                            AOT.mult)
                with tc.tile_wait_until(0.0202 + 0.0004 * c):
                    for k in range(ka, kb):
                        nc.tensor.matmul(TS[:, :f], WT[:, 0:128],
                                         G[:, f * k: f * (k + 1)],
                                         start=False,
                                         stop=(k == BINS - 2))

            # trust = tsum * (1/m)
            nc.vector.tensor_tensor(OUT[:], TS[:, :f], REC[:], AOT.mult)
            nc.sync.dma_start(d_out, OUT[:])

    nc.compile()
    return nc


_CACHE: dict = {}


def _get_nc(beta: float, mq: float):
    key = (beta, mq)
    if key not in _CACHE:
        _CACHE[key] = build_nc(beta, mq)
    return _CACHE[key]


def make_in_maps(inptasksperf, difficulties_obs, difficulties_pred,
                 n_total=N_TOTAL, ncores=NCORES, n_pad=N_PAD, p=P):
    """Host-side recode: bucket(d_obs) + perf flags -> (lo, hi) int8 clamp
    params, shard + pad + t-inner relayout."""
    perf = np.asarray(inptasksperf)
    dobs = np.asarray(difficulties_obs, dtype=np.float32)[..., 0]    # [T, N]
    dpred = np.asarray(difficulties_pred, dtype=np.float32)[..., 0]  # [N]
    f = n_pad // p
    nc_n = n_total // ncores
    steps = _steps_np()

    # b = #{k : s_k < d} in 0..11 (exact f32 comparisons, matches the
    # reference's mask since bucketing commutes with the min/max fold)
    b = np.searchsorted(steps, dobs.ravel(), side="left").astype(
        np.int8).reshape(dobs.shape)
    p0 = perf[..., 0] != 0
    p1 = perf[..., 1] != 0
    suc = p1 & ~p0
    lo = np.where(suc, b, 0).astype(np.int8)
    hi = np.where(p0, b, np.int8(15)).astype(np.int8)
    # slot-0 self-reset: state after step 0 is exactly v0
    v0 = np.where(suc[0], b[0], 0).astype(np.int8)
    lo[0] = v0
    hi[0] = v0

    in_maps = []
    for c in range(ncores):
        sl = slice(c * nc_n, (c + 1) * nc_n)
        lop = np.zeros((T, n_pad), np.int8)
        lop[:, :nc_n] = lo[:, sl]
        hip = np.zeros((T, n_pad), np.int8)
        hip[:, :nc_n] = hi[:, sl]
        loc = lop.reshape(T, p, f).transpose(1, 2, 0)   # [p, f, T]
        hic = hip.reshape(T, p, f).transpose(1, 2, 0)
        # tile-major pack: [lo-tile | hi-tile] contiguous per scan tile
        blocks = []
        fa = 0
        for ftj in FT:
            blocks.append(loc[:, fa:fa + ftj, :].reshape(p, ftj * T))
            blocks.append(hic[:, fa:fa + ftj, :].reshape(p, ftj * T))
            fa += ftj
        wp = np.ascontiguousarray(np.concatenate(blocks, axis=1))  # [p,2fT]

        dpc = np.zeros((n_pad,), np.float32)
        dpc[:nc_n] = dpred[sl]
        in_maps.append({"wp": wp, "dpred": dpc})
    return in_maps


def make_consts(beta, p=P):
    """Identity (bf16) for the PE PSUM-accumulate, plus activation bias
    consts."""
    wm = np.eye(p, 128, dtype=np.float32)
    steps = _steps_np()
    cb0 = np.float32(-(np.float64(beta) * np.float64(steps[0])))
    cb = np.broadcast_to(np.array([cb0, np.float32(BINS)], np.float32),
                         (p, 2))
    return {"wmats": wm.astype(ml_dtypes.bfloat16),
            "cbias": np.ascontiguousarray(cb)}


def kernel(inptasksobs=None, inptasksperf=None, inptaskspred=None,
           num_obs_tasks=None, tasksobsids=None, taskspredids=None,
           difficulties_obs=None, difficulties_pred=None,
           betas=None, zetas=None, **_):
    beta = float(np.float32(np.asarray(betas).reshape(-1)[0]))
    zeta = np.float32(np.asarray(zetas).reshape(-1)[0])
    mq = float(np.float32(-(zeta * zeta)))

    nc = _get_nc(beta, mq)
    in_maps = make_in_maps(inptasksperf, difficulties_obs, difficulties_pred)
    consts = make_consts(beta)
    for m in in_maps:
        m.update(consts)
    res = bass_utils.run_bass_kernel_spmd(nc, in_maps,
                                          core_ids=list(range(NCORES)))
    nc_n = N_TOTAL // NCORES
    parts = [np.asarray(r["out"]).reshape(-1)[:nc_n] for r in res.results]
    return np.concatenate(parts).reshape(N_TOTAL, 1).astype(np.float32)


if __name__ == "__main__":
    rng = np.random.default_rng(0)
    ins = {
        "inptasksperf": rng.integers(0, 2, (T, N_TOTAL, 2)).astype(np.int32),
        "difficulties_obs": (0.9 * rng.random((T, N_TOTAL, 1))).astype(np.float32),
        "difficulties_pred": (0.9 * rng.random((N_TOTAL, 1))).astype(np.float32),
        "betas": np.array([7.0], np.float32),
        "zetas": np.array([0.5], np.float32),
    }
    out = kernel(**ins)
    print(out.shape, out.dtype, out[:5, 0])
